# revision 1
# baseline (speedup 1.0000x reference)
"""KnowledgeAwareAttention Trainium2 kernel (8-core SPMD, row-sharded).

attn[i,j] = sum_d R_emb[q[i,j],d] * x[j,d] * x[i,d]
out = softmax(attn, -1) @ x

Strategy per core (128 output rows):
  - PE computes 42 relation "planes" T_k = (x_I * R_k) @ x^T  [128,1024]
    (contraction over d=256 in two 128-chunks, PSUM-accumulated).
  - The per-element selection attn[i,j] = T_{q[i,j]}[i,j] is a binary mux
    tree over the 6 bits of q: 21 ScalarE PSUM->SBUF copies (even planes)
    + 41 VectorE copy_predicated merges with host-precomputed bit masks.
  - softmax without max-subtraction (scores are tiny: |attn| < ~0.2),
    exp on ScalarE with fused row-sum (accum_out), reciprocal on VectorE.
  - output matmul: 8 PE transposes of the exp-plane + 8 accumulating
    matmuls against x chunks; final row-scale by 1/Z fused into the
    PSUM->SBUF copy on ScalarE.
Inputs are sharded/prepared on host: q bit-planes as f32 masks, x^T,
x^T block columns, R^T (all f32).
"""

import numpy as np

import concourse.bass as bass
import concourse.mybir as mybir
import concourse.tile as tile
from concourse.bass_utils import run_bass_kernel_spmd
from concourse.masks import make_identity

B = 1024
D = 256
NREL = 42
NCORES = 8
P = 128  # rows per core
F32 = mybir.dt.float32
AF = mybir.ActivationFunctionType


def _patch_tile_tail_drain():
    """This container's walrus rejects >1 sync-wait command on the
    kernel-tail SP Drain. Split the waits across SP nops."""
    import concourse.mybir as mybir_
    import concourse.tile as tile_

    def _drain_and_barrier(self, tick_clock, wait_clock):
        nc = self.nc
        drain_inst = nc.sync.drain()
        wait_clock.add_sem_waits(
            drain_inst.ins, tile_.ScopedClock({None: tick_clock.global_clock})
        )
        si = drain_inst.ins.sync_info
        waits = list(si.on_wait) if si and si.on_wait else []
        if len(waits) > 1:
            si.on_wait = waits[:1]
            for w in waits[1:]:
                nop = nc.sync.nop(nofuse=True)
                nop.ins.sync_info = mybir_.SyncInfo(on_wait=[w], on_update=[])
        nc.all_engine_barrier()
        assert self.sems is not None
        popped = nc._tile_sem_poison_stack.pop()
        assert popped is self._sem_poison
        nc.clear_and_free_semaphores(list(self.sems.allocated().values()))
        nc.all_engine_barrier()

    tile_.TileContext._drain_and_barrier = _drain_and_barrier


_patch_tile_tail_drain()


_MAX_WAITS = 1


def _split_excess_waits(nc: bass.Bass, max_waits: int = _MAX_WAITS) -> None:
    """This container's walrus caps the number of sync-wait commands one
    instruction may carry. Move excess waits onto same-engine NoOps
    inserted immediately before the instruction."""
    cnt = 0
    for wrapper in nc.bb_map.values():
        bb = wrapper.bb
        old = list(bb.instructions)
        new = []
        changed = False
        for ins in old:
            si = ins.sync_info
            waits = list(si.on_wait) if si and si.on_wait else []
            if len(waits) > max_waits:
                changed = True
                si.on_wait = waits[:max_waits]
                rest = waits[max_waits:]
                for i in range(0, len(rest), max_waits):
                    nop = mybir.InstNoOp(name=f"waitnop{cnt}", ins=[], outs=[])
                    cnt += 1
                    nop.engine = ins.engine
                    nop.sync_info = mybir.SyncInfo(
                        on_wait=rest[i:i + max_waits], on_update=[]
                    )
                    new.append(nop)
            new.append(ins)
        if changed:
            bb.instructions = new


def build_nc() -> bass.Bass:
    nc = bass.Bass()
    xT_d = nc.dram_tensor("xt", [P, 2 * B], F32, kind="ExternalInput")
    xTI_d = nc.dram_tensor("xti", [P, 2 * P], F32, kind="ExternalInput")
    x_d = nc.dram_tensor("x", [P, 8 * D], F32, kind="ExternalInput")
    rt_d = nc.dram_tensor("rt", [P, 2 * NREL], F32, kind="ExternalInput")
    bits_d = nc.dram_tensor("bits", [P, 6 * B], mybir.dt.int8, kind="ExternalInput")
    out_d = nc.dram_tensor("out", [P, D], F32, kind="ExternalOutput")

    with tile.TileContext(nc) as tc:
        with (
            tc.tile_pool(name="const", bufs=1) as const,
            tc.tile_pool(name="lhs", bufs=12) as lhsp,
            tc.tile_pool(name="planes", bufs=1) as planep,
            tc.tile_pool(name="sm", bufs=1) as smp,
            tc.tile_pool(name="et", bufs=4) as etp,
        ):
            # ---- loads (coalesced: one wide DMA per logical input) ----
            xT_t = const.tile([P, 2 * B], F32, tag="xt", name="xt_t")
            xTI_t = const.tile([P, 2 * P], F32, tag="xti", name="xti_t")
            rt_t = const.tile([P, 2 * NREL], F32, tag="rt", name="rt_t")
            bits_t = const.tile([P, 6 * B], mybir.dt.int8, tag="bits", name="bits_t")
            xc_t = const.tile([P, 8 * D], F32, tag="xc", name="xc_t")
            nc.sync.dma_start(xT_t[:, :], xT_d[:, :])
            nc.sync.dma_start(xTI_t[:, :], xTI_d[:, :])
            nc.sync.dma_start(rt_t[:, :], rt_d[:, :])
            nc.sync.dma_start(bits_t[:, :], bits_d[:, :])
            nc.sync.dma_start(xc_t[:, :], x_d[:, :])
            xT = [xT_t[:, c * B:(c + 1) * B] for c in range(2)]
            xTI = [xTI_t[:, c * P:(c + 1) * P] for c in range(2)]
            rt = [rt_t[:, c * NREL:(c + 1) * NREL] for c in range(2)]
            bits = [bits_t[:, l * B:(l + 1) * B] for l in range(6)]
            xc = [xc_t[:, j * D:(j + 1) * D] for j in range(8)]
            ident = const.tile([P, P], F32, tag="ident")
            make_identity(nc, ident[:, :])

            # ---- phase B: 42 planes + tree level 0 ----
            planes = []
            with tc.tile_pool(name="pp", bufs=4, space="PSUM") as pp:
                for m in range(21):
                    P_m = None
                    for parity in (0, 1):
                        k = 2 * m + parity
                        pt = pp.tile([P, B], F32, tag="plane", name=f"t{k}")
                        for c in range(2):
                            lh = lhsp.tile([P, P], F32, tag="lh", name=f"lh{k}_{c}")
                            nc.gpsimd.tensor_scalar_mul(
                                lh[:, :], xTI[c], rt[c][:, k:k + 1]
                            )
                            for jh in range(2):
                                nc.tensor.matmul(
                                    pt[:, jh * 512:(jh + 1) * 512],
                                    lhsT=lh[:, :],
                                    rhs=xT[c][:, jh * 512:(jh + 1) * 512],
                                    start=(c == 0),
                                    stop=(c == 1),
                                )
                        if parity == 0:
                            P_m = planep.tile([P, B], F32, tag=f"p{m}", name=f"p{m}")
                            nc.scalar.copy(P_m[:, :], pt[:, :])
                        else:
                            nc.vector.copy_predicated(
                                P_m[:, :], bits[0], pt[:, :]
                            )
                    planes.append(P_m)

            # ---- phase C: tree levels 1..5 ----
            lvl_planes = planes
            for lvl in range(1, 6):
                nxt = []
                for m in range(len(lvl_planes) // 2):
                    a, b = lvl_planes[2 * m], lvl_planes[2 * m + 1]
                    nc.vector.copy_predicated(a[:, :], bits[lvl], b[:, :])
                    nxt.append(a)
                if len(lvl_planes) % 2:
                    nxt.append(lvl_planes[-1])
                lvl_planes = nxt
            attn = lvl_planes[0]

            # ---- phase D: exp + row sums ----
            E = smp.tile([P, B], F32, tag="E")
            z = smp.tile([P, 1], F32, tag="z")
            rz = smp.tile([P, 1], F32, tag="rz")
            nc.scalar.activation(E[:, :], attn[:, :], AF.Exp, accum_out=z[:, :])
            nc.vector.reciprocal(rz[:, :], z[:, :])

            # ---- phase E: transposes + output matmul ----
            with (
                tc.tile_pool(name="tp", bufs=2, space="PSUM") as tp,
                tc.tile_pool(name="op", bufs=1, space="PSUM") as op,
            ):
                out_ps = op.tile([P, D], F32, tag="out")
                for jc in range(8):
                    ptile = tp.tile([P, P], F32, tag="tp", name=f"tp{jc}")
                    nc.tensor.transpose(ptile[:, :], E[:, jc * P:(jc + 1) * P], ident[:, :])
                    et = etp.tile([P, P], F32, tag="et", name=f"et{jc}")
                    nc.scalar.copy(et[:, :], ptile[:, :])
                    nc.tensor.matmul(
                        out_ps[:, :],
                        lhsT=et[:, :],
                        rhs=xc[jc],
                        start=(jc == 0),
                        stop=(jc == 7),
                    )
                # ---- phase F: scale rows by 1/Z and store ----
                out_sb = smp.tile([P, D], F32, tag="osb")
                nc.scalar.activation(out_sb[:, :], out_ps[:, :], AF.Copy, scale=rz[:, :])
                nc.sync.dma_start(out_d[:, :], out_sb[:, :])
    _split_excess_waits(nc)
    return nc


_NC_CACHE = None


def _get_nc():
    global _NC_CACHE
    if _NC_CACHE is None:
        _NC_CACHE = build_nc()
    return _NC_CACHE


def make_in_maps(x, q, R):
    x = np.asarray(x, dtype=np.float32)
    q = np.asarray(q)
    R = np.asarray(R, dtype=np.float32)

    xT = np.ascontiguousarray(x.T)                      # [D, B]
    rt = np.ascontiguousarray(R.T)                      # [D, 42]
    q32 = q.astype(np.int32)

    # packed [128, W] layouts: d-chunks side by side along the free axis
    xt_p = np.ascontiguousarray(
        xT.reshape(2, P, B).transpose(1, 0, 2).reshape(P, 2 * B))
    rt_p = np.ascontiguousarray(
        rt.reshape(2, P, NREL).transpose(1, 0, 2).reshape(P, 2 * NREL))
    x_p = np.ascontiguousarray(
        x.reshape(8, P, D).transpose(1, 0, 2).reshape(P, 8 * D))

    in_maps = []
    for c in range(NCORES):
        rows = slice(c * P, (c + 1) * P)
        qb = q32[rows]                                   # [128, B]
        bits = np.empty((P, 6 * B), dtype=np.int8)
        for l in range(6):
            bits[:, l * B:(l + 1) * B] = ((qb >> l) & 1).astype(np.int8)
        xti = xT[:, rows]                                # [D, 128]
        xti_p = np.ascontiguousarray(
            xti.reshape(2, P, P).transpose(1, 0, 2).reshape(P, 2 * P))
        in_maps.append(
            {
                "xt": xt_p,
                "xti": xti_p,
                "x": x_p,
                "rt": rt_p,
                "bits": bits,
            }
        )
    return in_maps


def kernel(x, x_mask, q, f, R_emb):
    in_maps = make_in_maps(x, q, R_emb)
    res = run_bass_kernel_spmd(_get_nc(), in_maps, core_ids=list(range(NCORES)))
    out = np.concatenate([res.results[c]["out"] for c in range(NCORES)], axis=0)
    return out



# revision 8
# speedup vs baseline: 3.1010x; 3.1010x over previous
"""KnowledgeAwareAttention Trainium2 kernel (8-core SPMD, row-sharded).

attn[i,j] = sum_d R_emb[q[i,j],d] * x[j,d] * x[i,d]
out = softmax(attn, -1) @ x

Per core (128 output rows):
  - DVE prep: lh[c][p, k*128+i] = x[i, c*128+p] * R[k, c*128+p] in bf16 via
    two broadcast-AP tensor_tensor ops (stride-0 repeats, no gpsimd).
  - PE computes 42 relation planes T_k = (x_I * R_k) @ x^T [128,1024] in bf16
    (contraction d=256 as two 128-chunks, PSUM-accumulated).
  - Selection attn[i,j] = T_{q[i,j]}[i,j] is a 6-bit binary mux tree:
    L0 pairs are split between two styles to balance ScalarE/VectorE:
      style B: ScalarE evacuates both planes to SBUF bf16, DVE merges with a
               bf16 mask (2x DVE rate);
      style A: ScalarE evacuates even plane, DVE copy_predicated odd straight
               from PSUM (int8 mask).
    L1..L5 merges run on DVE in bf16 with bf16 masks, emitted eagerly.
  - softmax without max-subtraction (|attn| < ~0.2): Exp on ScalarE with fused
    row-sum accum, reciprocal on VectorE.
  - output matmul in bf16: 8 PE transposes of E + 8 accumulating matmuls
    against x chunks; row-scale by 1/Z fused into the PSUM->SBUF copy.
"""

import numpy as np
import ml_dtypes

import concourse.bass as bass
import concourse.mybir as mybir
import concourse.tile as tile
from concourse.bass_utils import run_bass_kernel_spmd
from concourse.masks import make_identity

B = 1024
D = 256
NREL = 42
NCORES = 8
P = 128  # rows per core
NPAIR = 21
F32 = mybir.dt.float32
BF16 = mybir.dt.bfloat16
AF = mybir.ActivationFunctionType

# L0 pairs where ScalarE evacuates both planes and DVE merges in SBUF (bf16
# 2x). Remaining pairs: DVE merges the odd plane straight from PSUM.
STYLE_B = set(m for m in range(NPAIR) if m % 3 != 2)  # 14 of 21


def _patch_tile_tail_drain():
    """This container's walrus rejects >1 sync-wait command on the
    kernel-tail SP Drain. Split the waits across SP nops."""
    import concourse.mybir as mybir_
    import concourse.tile as tile_

    def _drain_and_barrier(self, tick_clock, wait_clock):
        nc = self.nc
        drain_inst = nc.sync.drain()
        wait_clock.add_sem_waits(
            drain_inst.ins, tile_.ScopedClock({None: tick_clock.global_clock})
        )
        si = drain_inst.ins.sync_info
        waits = list(si.on_wait) if si and si.on_wait else []
        if len(waits) > 1:
            si.on_wait = waits[:1]
            for w in waits[1:]:
                nop = nc.sync.nop(nofuse=True)
                nop.ins.sync_info = mybir_.SyncInfo(on_wait=[w], on_update=[])
        nc.all_engine_barrier()
        assert self.sems is not None
        popped = nc._tile_sem_poison_stack.pop()
        assert popped is self._sem_poison
        nc.clear_and_free_semaphores(list(self.sems.allocated().values()))
        nc.all_engine_barrier()

    tile_.TileContext._drain_and_barrier = _drain_and_barrier


_patch_tile_tail_drain()


_MAX_WAITS = 1


def _split_excess_waits(nc: bass.Bass, max_waits: int = _MAX_WAITS) -> None:
    """This container's walrus caps the number of sync-wait commands one
    instruction may carry. Move excess waits onto same-engine NoOps
    inserted immediately before the instruction."""
    cnt = 0
    for wrapper in nc.bb_map.values():
        bb = wrapper.bb
        old = list(bb.instructions)
        new = []
        changed = False
        for ins in old:
            si = ins.sync_info
            waits = list(si.on_wait) if si and si.on_wait else []
            if len(waits) > max_waits:
                changed = True
                si.on_wait = waits[:max_waits]
                rest = waits[max_waits:]
                for i in range(0, len(rest), max_waits):
                    nop = mybir.InstNoOp(name=f"waitnop{cnt}", ins=[], outs=[])
                    cnt += 1
                    nop.engine = ins.engine
                    nop.sync_info = mybir.SyncInfo(
                        on_wait=rest[i:i + max_waits], on_update=[]
                    )
                    new.append(nop)
            new.append(ins)
        if changed:
            bb.instructions = new


def build_nc() -> bass.Bass:
    nc = bass.Bass()
    xT_d = nc.dram_tensor("xt", [P, 2 * B], BF16, kind="ExternalInput")
    xTI_d = nc.dram_tensor("xti", [P, 2 * P], BF16, kind="ExternalInput")
    rt_d = nc.dram_tensor("rt", [P, 2 * NREL], BF16, kind="ExternalInput")
    b0_d = nc.dram_tensor("b0", [P, B], mybir.dt.int8, kind="ExternalInput")
    bb_d = nc.dram_tensor("bb", [P, 6 * B], mybir.dt.uint16, kind="ExternalInput")
    x_d = nc.dram_tensor("x", [P, 8 * D], BF16, kind="ExternalInput")
    out_d = nc.dram_tensor("out", [P, D], F32, kind="ExternalOutput")

    with tile.TileContext(nc) as tc:
        with (
            tc.tile_pool(name="const", bufs=1) as const,
            tc.tile_pool(name="lh", bufs=1) as lhp,
            tc.tile_pool(name="planes", bufs=1) as planep,
            tc.tile_pool(name="po", bufs=3) as pop,
            tc.tile_pool(name="sm", bufs=1) as smp,
            tc.tile_pool(name="et", bufs=4) as etp,
        ):
            # ---- loads ----
            xT_t = const.tile([P, 2 * B], BF16, tag="xt", name="xt_t")
            xTI_t = const.tile([P, 2 * P], BF16, tag="xti", name="xti_t")
            rt_t = const.tile([P, 2 * NREL], BF16, tag="rt", name="rt_t")
            b0_t = const.tile([P, B], mybir.dt.int8, tag="b0", name="b0_t")
            bb_t = const.tile([P, 6 * B], mybir.dt.uint16, tag="bb", name="bb_t")
            xc_t = const.tile([P, 8 * D], BF16, tag="xc", name="xc_t")
            nc.sync.dma_start(xTI_t[:, :], xTI_d[:, :])
            nc.sync.dma_start(rt_t[:, :], rt_d[:, :])
            nc.sync.dma_start(xT_t[:, :], xT_d[:, :])
            nc.sync.dma_start(b0_t[:, :], b0_d[:, :])
            nc.sync.dma_start(bb_t[:, :], bb_d[:, :])
            nc.sync.dma_start(xc_t[:, :], x_d[:, :])
            xT = [xT_t[:, c * B:(c + 1) * B] for c in range(2)]
            bit_bf = [bb_t[:, l * B:(l + 1) * B] for l in range(6)]
            xc = [xc_t[:, j * D:(j + 1) * D] for j in range(8)]
            ident = const.tile([P, P], BF16, tag="ident")
            make_identity(nc, ident[:, :])

            # ---- prep: lh[c][p, k*128+i] = xTI[c][p,i] * rt[c][p,k] ----
            lh = []
            for c in range(2):
                lh_c = lhp.tile([P, NREL * P], BF16, tag=f"lh{c}", name=f"lh{c}")
                in0 = (
                    xTI_t[:, c * P:(c + 1) * P]
                    .unsqueeze(1)
                    .broadcast_to([P, NREL, P])
                )
                in1 = (
                    rt_t[:, c * NREL:(c + 1) * NREL]
                    .unsqueeze(2)
                    .broadcast_to([P, NREL, P])
                )
                outv = lh_c[:, :].rearrange("p (k i) -> p k i", k=NREL)
                nc.vector.tensor_tensor(outv, in0, in1, mybir.AluOpType.mult)
                lh.append(lh_c)

            # ---- planes + eager mux tree ----
            # levels[l] = list of (index, tile) ready at level l
            levels = [[] for _ in range(7)]
            counts = [NPAIR, 11, 6, 3, 2, 1]

            def try_merge(l):
                if l >= len(counts):
                    return
                # merge adjacent ready planes at level l with mask bit l+1
                while len(levels[l]) >= 2:
                    (ia, a), (ib, b) = levels[l][0], levels[l][1]
                    if ia % 2 == 0 and ib == ia + 1:
                        nc.vector.copy_predicated(
                            a[:, :], bit_bf[l + 1], b[:, :]
                        )
                        levels[l] = levels[l][2:]
                        levels[l + 1].append((ia // 2, a))
                        try_merge(l + 1)
                    elif ia == counts[l] - 1 and counts[l] % 2 == 1:
                        # odd leftover promotes unchanged
                        levels[l] = levels[l][1:]
                        levels[l + 1].append((ia // 2, a))
                        try_merge(l + 1)
                    else:
                        break
                # single leftover at an odd tail also promotes
                if (
                    len(levels[l]) == 1
                    and levels[l][0][0] == counts[l] - 1
                    and counts[l] % 2 == 1
                ):
                    ia, a = levels[l][0]
                    levels[l] = []
                    levels[l + 1].append((ia // 2, a))
                    try_merge(l + 1)

            with tc.tile_pool(name="pp", bufs=4, space="PSUM") as pp:
                for m in range(NPAIR):
                    pts = []
                    for parity in (0, 1):
                        k = 2 * m + parity
                        pt = pp.tile([P, B], F32, tag="plane", name=f"t{k}")
                        for c in range(2):
                            lhs = lh[c][:, k * P:(k + 1) * P]
                            for jh in range(2):
                                nc.tensor.matmul(
                                    pt[:, jh * 512:(jh + 1) * 512],
                                    lhsT=lhs,
                                    rhs=xT[c][:, jh * 512:(jh + 1) * 512],
                                    start=(c == 0),
                                    stop=(c == 1),
                                )
                        pts.append(pt)
                    P_m = planep.tile([P, B], BF16, tag=f"p{m}", name=f"p{m}")
                    nc.scalar.copy(P_m[:, :], pts[0][:, :])
                    if m in STYLE_B:
                        Po = pop.tile([P, B], BF16, tag="po", name=f"po{m}")
                        nc.scalar.copy(Po[:, :], pts[1][:, :])
                        nc.vector.copy_predicated(
                            P_m[:, :], bit_bf[0], Po[:, :]
                        )
                    else:
                        nc.vector.copy_predicated(
                            P_m[:, :], b0_t[:, :], pts[1][:, :]
                        )
                    levels[0].append((m, P_m))
                    try_merge(0)

            assert len(levels[6]) == 1, [len(lv) for lv in levels]
            attn = levels[6][0][1]

            # ---- exp + row sums (bf16 out, fp32 accum) ----
            Ebf = smp.tile([P, B], BF16, tag="Ebf")
            z = smp.tile([P, 1], F32, tag="z")
            rz = smp.tile([P, 1], F32, tag="rz")
            nc.scalar.activation(Ebf[:, :], attn[:, :], AF.Exp, accum_out=z[:, :])
            nc.vector.reciprocal(rz[:, :], z[:, :])

            # ---- transposes + output matmul (bf16) ----
            with (
                tc.tile_pool(name="tp", bufs=2, space="PSUM") as tp,
                tc.tile_pool(name="op", bufs=1, space="PSUM") as op,
            ):
                out_ps = op.tile([P, D], F32, tag="out")
                for jc in range(8):
                    ptile = tp.tile([P, P], BF16, tag="tp", name=f"tp{jc}")
                    nc.tensor.transpose(
                        ptile[:, :], Ebf[:, jc * P:(jc + 1) * P], ident[:, :]
                    )
                    et = etp.tile([P, P], BF16, tag="et", name=f"et{jc}")
                    nc.scalar.copy(et[:, :], ptile[:, :])
                    nc.tensor.matmul(
                        out_ps[:, :],
                        lhsT=et[:, :],
                        rhs=xc[jc],
                        start=(jc == 0),
                        stop=(jc == 7),
                    )
                # ---- scale rows by 1/Z and store ----
                out_sb = smp.tile([P, D], F32, tag="osb")
                nc.scalar.activation(
                    out_sb[:, :], out_ps[:, :], AF.Copy, scale=rz[:, :]
                )
                nc.sync.dma_start(out_d[:, :], out_sb[:, :])
    _split_excess_waits(nc)
    return nc


_NC_CACHE = None


def _get_nc():
    global _NC_CACHE
    if _NC_CACHE is None:
        _NC_CACHE = build_nc()
    return _NC_CACHE


def make_in_maps(x, q, R):
    x = np.asarray(x, dtype=np.float32)
    q = np.asarray(q)
    R = np.asarray(R, dtype=np.float32)
    bf = ml_dtypes.bfloat16

    xT = np.ascontiguousarray(x.T)                      # [D, B]
    rt = np.ascontiguousarray(R.T)                      # [D, 42]
    q32 = q.astype(np.int32)

    # packed [128, W] layouts: d-chunks side by side along the free axis
    xt_p = np.ascontiguousarray(
        xT.reshape(2, P, B).transpose(1, 0, 2).reshape(P, 2 * B)).astype(bf)
    rt_p = np.ascontiguousarray(
        rt.reshape(2, P, NREL).transpose(1, 0, 2).reshape(P, 2 * NREL)
    ).astype(bf)
    x_p = np.ascontiguousarray(
        x.reshape(8, P, D).transpose(1, 0, 2).reshape(P, 8 * D)).astype(bf)

    in_maps = []
    for c in range(NCORES):
        rows = slice(c * P, (c + 1) * P)
        qb = q32[rows]                                   # [128, B]
        b0 = ((qb >> 0) & 1).astype(np.int8)
        bb = np.empty((P, 6 * B), dtype=np.uint16)
        for l in range(6):
            bb[:, l * B:(l + 1) * B] = ((qb >> l) & 1).astype(np.uint16)
        xti = xT[:, rows]                                # [D, 128]
        xti_p = np.ascontiguousarray(
            xti.reshape(2, P, P).transpose(1, 0, 2).reshape(P, 2 * P)
        ).astype(bf)
        in_maps.append(
            {
                "xt": xt_p,
                "xti": xti_p,
                "x": x_p,
                "rt": rt_p,
                "b0": b0,
                "bb": bb,
            }
        )
    return in_maps


def kernel(x, x_mask, q, f, R_emb):
    in_maps = make_in_maps(x, q, R_emb)
    res = run_bass_kernel_spmd(_get_nc(), in_maps, core_ids=list(range(NCORES)))
    out = np.concatenate([res.results[c]["out"] for c in range(NCORES)], axis=0)
    return out
